# revision 1
# baseline (speedup 1.0000x reference)
"""Trainium2 Bass kernel for the Chambolle-Pock-style primal/dual stencil loop.

Math (per image, H=W=1024, EPS=0.5, TAU=0.5, 10 iterations):
    u = sigmoid(o/EPS); q = 0
    repeat 10x:
        q  = relu(q - TAU*(vf1*Dy(u) + vf0*Dx(u)))   # forward diffs, zero pad
        Tq = BDy(vf1*q) + BDx(vf0*q)                  # backward diffs, zero pad
        u  = sigmoid((o - Tq)/EPS)
    return (o - Tq)/EPS

Rescaling trick: with qh = 2*sqrt(2)*q, g = vf/sqrt(2) (host-side) and
s = 2(o - Tq), and representing u through t = tanh(s/2) (u = 0.5 + 0.5t, the
0.5s cancel in every stencil difference; zero-padding of u becomes
(-1)-padding of t):
    qh = relu(qh - (g1*Dy(t) + g0*Dx(t)))            # t pads: -1
    s  = o2 - BDy(g1*qh) - BDx(g0*qh)                # o2 = 2*o, pads 0
    t  = tanh(s/2)
and the final output is s itself.  tanh is used instead of sigmoid because its
activation table is ~10x more accurate (4 vs 40 ULP) and the relu makes
isolated pixels chaotic under any per-step rounding noise; everything else is
kept in exact fp32 for the same reason (measured: rel-L2 vs the fp32 jax
reference is ~1e-5, max-abs ~0.028 — the fp32 reference's own fp64 envelope).

Sharding: pure data parallel, one image per NeuronCore (B=8 over 8 cores),
vf0/vf1 broadcast to all cores.

Layout: image row y = 8*p + i -> partition p (0..127), plane i (0..7) in the
free dimension.  A +1 row shift is then a free-dim plane offset for i<7; only
the plane-7 -> next-partition boundary needs a cross-partition move, done with
a tiny SBUF->SBUF DMA into a 9th plane.  Column shifts use guard columns.
The whole working set (6 image buffers, ~200KB/partition) stays SBUF resident,
so HBM traffic is one 12MB load + 4MB store per core.  All elementwise ops run
on VectorE split into plane-halves so relu/tanh (ScalarE) and the boundary
DMAs overlap the VectorE stream.
"""

import numpy as np

import concourse.bacc as bacc
import concourse.mybir as mybir
from concourse.tile import TileContext
from concourse import bass_utils

F32 = mybir.dt.float32
AF = mybir.ActivationFunctionType

B, H, W = 8, 1024, 1024
P = 128          # SBUF partitions
NP = H // P      # planes per partition = 8
WG = W + 1       # plane width incl. one guard column
MAXITER = 10

_CACHE = {}
LAST_RESULTS = None  # BassKernelResults of the most recent run (for test.py)


def _build(reps=1):
    """Build the Bass program.  reps>1 repeats the whole computation (state
    re-initialized each rep, same output) — used only for wall-clock timing
    of the HW kernel when no NTFF profiling is available."""
    nc = bacc.Bacc("TRN2", target_bir_lowering=False, debug=False)

    o2_d = nc.dram_tensor("o2", [H, W], F32, kind="ExternalInput").ap()
    g0_d = nc.dram_tensor("g0", [H, W], F32, kind="ExternalInput").ap()
    g1_d = nc.dram_tensor("g1", [H, W], F32, kind="ExternalInput").ap()
    out_d = nc.dram_tensor("out", [H, W], F32, kind="ExternalOutput").ap()

    # (H, W) -> (p, i, x) with y = 8*p + i
    o2_v = o2_d.rearrange("(p i) x -> p i x", i=NP)
    g0_v = g0_d.rearrange("(p i) x -> p i x", i=NP)
    g1_v = g1_d.rearrange("(p i) x -> p i x", i=NP)
    out_v = out_d.rearrange("(p i) x -> p i x", i=NP)

    v = nc.vector
    act = nc.scalar

    with TileContext(nc) as tc:
        with tc.tile_pool(name="main", bufs=1) as pool:
            o2t = pool.tile([P, NP, W], F32)
            g0t = pool.tile([P, NP, W], F32)
            g1t = pool.tile([P, NP, W], F32)
            qht = pool.tile([P, NP, W], F32)
            # su: planes 0..7 = t/s data (col W = -1 guard for x+1 reads),
            # plane 8 = boundary row t[8p+8, x] (partition 127 stays -1)
            sut = pool.tile([P, NP + 1, WG], F32)
            # tmp: planes 1..8 = a/b scratch at cols 1..W (col 0 = zero guard
            # for x-1 reads), plane 0 = boundary row a[8p-1, x]
            tmpt = pool.tile([P, NP + 1, WG], F32)

            halves = [(0, NP // 2), (NP // 2, NP)]

            def u_(lo, hi):
                return sut[:, lo:hi, 0:W]

            def unr(lo, hi):   # t[y+1, x] (plane 8 = boundary)
                return sut[:, lo + 1 : hi + 1, 0:W]

            def unc(lo, hi):   # t[y, x+1] (col W = -1 guard)
                return sut[:, lo:hi, 1 : W + 1]

            def t_(lo, hi):
                return tmpt[:, lo + 1 : hi + 1, 1 : W + 1]

            def tpr(lo, hi):   # a[y-1, x] (plane 0 = boundary)
                return tmpt[:, lo:hi, 1 : W + 1]

            def tpc(lo, hi):   # b[y, x-1] (col 0 = zero guard)
                return tmpt[:, lo + 1 : hi + 1, 0:W]

            def o2_(lo, hi):
                return o2t[:, lo:hi, :]

            def g0_(lo, hi):
                return g0t[:, lo:hi, :]

            def g1_(lo, hi):
                return g1t[:, lo:hi, :]

            def qh_(lo, hi):
                return qht[:, lo:hi, :]

            # --- setup ---
            # t-state guards are -1 (tanh representation of u=0 padding).
            v.memset(sut[:, :, :], -1.0)
            v.memset(tmpt[:, :, :], 0.0)  # zero guards + a-boundary row 0
            nc.sync.dma_start(out=o2t[:, :, :], in_=o2_v)
            nc.sync.dma_start(out=g0t[:, :, :], in_=g0_v)
            nc.sync.dma_start(out=g1t[:, :, :], in_=g1_v)

            def dma_ushift():
                # su[p, 8, x] = t[8p+8, x] = su[p+1, 0, x]; row 127 stays -1
                nc.sync.dma_start(
                    out=sut[0 : P - 1, NP, 0:W], in_=sut[1:P, 0, 0:W]
                )

            def dma_ashift():
                # tmp[p, 0, c] = a[8p-1] = tmp[p-1, 8, c]; row 0 stays 0
                nc.sync.dma_start(
                    out=tmpt[1:P, 0, 1 : W + 1], in_=tmpt[0 : P - 1, NP, 1 : W + 1]
                )

            for _rep in range(reps):
                if reps > 1:
                    v.memset(sut[:, :, :], -1.0)
                v.memset(qht[:, :, :], 0.0)
                for lo, hi in halves:
                    act.activation(u_(lo, hi), o2_(lo, hi), AF.Tanh, scale=0.5)
                dma_ushift()

                for it in range(MAXITER):
                    last = it == MAXITER - 1
                    # dual: qh = relu(qh - g1*Dy(t) - g0*Dx(t))
                    for lo, hi in halves:
                        v.tensor_sub(t_(lo, hi), unr(lo, hi), u_(lo, hi))
                        v.tensor_mul(t_(lo, hi), t_(lo, hi), g1_(lo, hi))
                        v.tensor_sub(qh_(lo, hi), qh_(lo, hi), t_(lo, hi))
                    for lo, hi in halves:
                        v.tensor_sub(t_(lo, hi), unc(lo, hi), u_(lo, hi))
                        v.tensor_mul(t_(lo, hi), t_(lo, hi), g0_(lo, hi))
                        v.tensor_sub(qh_(lo, hi), qh_(lo, hi), t_(lo, hi))
                        act.activation(qh_(lo, hi), qh_(lo, hi), AF.Relu)
                    # primal: s = o2 - (a-a_pr) - (b-b_pc), a = g1*qh, b = g0*qh
                    # upper a-half first so the boundary-row DMA fires early
                    v.tensor_mul(t_(*halves[1]), g1_(*halves[1]), qh_(*halves[1]))
                    dma_ashift()
                    v.tensor_mul(t_(*halves[0]), g1_(*halves[0]), qh_(*halves[0]))
                    for lo, hi in halves:
                        v.tensor_sub(u_(lo, hi), o2_(lo, hi), t_(lo, hi))
                        v.tensor_add(u_(lo, hi), u_(lo, hi), tpr(lo, hi))
                    for lo, hi in halves:
                        v.tensor_mul(t_(lo, hi), g0_(lo, hi), qh_(lo, hi))
                        v.tensor_sub(u_(lo, hi), u_(lo, hi), t_(lo, hi))
                        v.tensor_add(u_(lo, hi), u_(lo, hi), tpc(lo, hi))
                        if not last:
                            act.activation(
                                u_(lo, hi), u_(lo, hi), AF.Tanh, scale=0.5
                            )
                            if lo == 0:
                                dma_ushift()

            nc.sync.dma_start(out=out_v, in_=sut[:, 0:NP, 0:W])

    nc.compile()
    return nc


def kernel(o, vector_field, nabla_w, div_w):
    global LAST_RESULTS
    if "nc" not in _CACHE:
        _CACHE["nc"] = _build()
    nc = _CACHE["nc"]

    o2 = np.ascontiguousarray(2.0 * np.asarray(o, dtype=np.float32)[:, 0])
    vf = np.asarray(vector_field, dtype=np.float32)
    s = np.float32(1.0 / np.sqrt(2.0))
    g0 = np.ascontiguousarray(vf[:, :, 0] * s)
    g1 = np.ascontiguousarray(vf[:, :, 1] * s)

    in_maps = [{"o2": o2[b], "g0": g0, "g1": g1} for b in range(B)]
    res = bass_utils.run_bass_kernel_spmd(nc, in_maps, core_ids=list(range(B)))
    LAST_RESULTS = res
    return np.stack([r["out"] for r in res.results]).astype(np.float32)

